# revision 8
# baseline (speedup 1.0000x reference)
"""Trainium2 Bass kernel for the dual-branch cross-attention module.

Computation (see the module's reference):
    q1,k1,v1 = split(x @ w_qkv1); q2,k2,v2 = split(y @ w_qkv2)   (B,H,L,D)
    a1 = softmax(1 - q1 k2^T / sqrt(D));  xo = a1 @ v1
    a2 = softmax(1 - q2 k1^T / sqrt(D));  yo = a2 @ v2
    out = (xo @ w_p1 + b_p1, yo @ w_p2 + b_p2)

Sharding: batch*heads across 8 cores. Core c handles batch b=c//2 and the
8-head slice h0=(c%2)*8; the host sums the two partial projections per
batch and adds the bias (softmax(1-z) == softmax(-z): shift dropped).

v4 design notes (changes vs v3):
 - Start: input DMAs are chunked (xT by KC, wv1 by KC, wq1/wk2 per pair)
   and triggered on BOTH hardware DGE queues (sync + scalar).  The first
   PE work (q1/k2 pair-0 QKV and v lt0/1) accumulates chunk-OUTER so
   matmuls begin as soon as the first chunks land instead of waiting for
   whole tensors.
 - PSUM pools are split: scores get a dedicated 2-buf [128,1024] pool,
   filler groups (qkv halves / proj) a 2-buf [128,512] pool, and PV one
   [65,1024] tile per (pair,lw) group.  This removes the pool-rotation
   coupling that made score matmuls wait on the 1.1us exp ACTIVATE.
 - Output projection accumulates in SBUF per pair (vector add of each
   pair's [128,256] psum) and is emitted as fillers as soon as each
   pair's onorm is ready; output DMA per 128-row chunk follows the last
   pair.  This removes the serial 15us tail.
 - normalize: reciprocal on the [1,512] denominator row first, then
   gpsimd partition-broadcast of the reciprocal, then one multiply.
Self-contained: shapes/sharding hardcoded; imports only the system bass stack.
"""

import os
import sys
from contextlib import ExitStack

import numpy as np
import ml_dtypes

for _p in ("/opt/trn_rl_repo", os.path.expanduser("~/.axon_site/_ro/trn_rl_repo")):
    if os.path.isdir(_p) and _p not in sys.path:
        sys.path.insert(0, _p)

import concourse.tile as tile
from concourse import bacc, mybir
from concourse.bass_utils import run_bass_kernel_spmd

F32 = mybir.dt.float32
BF16 = mybir.dt.bfloat16
EXP = mybir.ActivationFunctionType.Exp

L = 1024          # sequence length
DIM = 1024        # model dim
D = 64            # head dim
SCALE = D ** -0.5
PROJ = 256        # projection out dim
NCORES = 8
PAIRS = 4         # head pairs per core (8 heads / 2)
KC = 8            # contraction chunks of 128 over DIM
MC = 8            # key-position chunks of 128 over L
LWIN = 512        # query-window (matmul free dim)
NLW = L // LWIN
LAG = 2           # attention pipeline: pv trails scores by LAG slots

W_NAMES = ("wq1", "wk1", "wv1", "wq2", "wk2", "wv2")


def _build_body(nc, tc, ins, outs, ctx):
    big = ctx.enter_context(tc.tile_pool(name="big", bufs=1))
    ep = ctx.enter_context(tc.tile_pool(name="ep", bufs=4))
    nrm = ctx.enter_context(tc.tile_pool(name="nrm", bufs=4))
    sc_ps = ctx.enter_context(tc.tile_pool(name="sc_ps", bufs=2, space="PSUM"))
    fl_ps = ctx.enter_context(tc.tile_pool(name="fl_ps", bufs=2, space="PSUM"))
    pv_ps = ctx.enter_context(tc.tile_pool(name="pv_ps", bufs=1, space="PSUM"))

    # ---- persistent SBUF tiles; DMA straight into final bf16 layouts ----
    xT = big.tile([128, KC, L], BF16, tag="xT")
    yT = big.tile([128, KC, L], BF16, tag="yT")
    # q/k weights pair-major [128, PAIRS, KC, 128]; v weights [128, KC, 512]
    w_bf = {}
    for nm in ("wq1", "wk1", "wq2", "wk2"):
        w_bf[nm] = big.tile([128, PAIRS, KC, 128], BF16, tag=nm, name=nm)
    for nm in ("wv1", "wv2"):
        w_bf[nm] = big.tile([128, KC, 512], BF16, tag=nm, name=nm)
    wp_bf = {nm: big.tile([128, PAIRS, PROJ], BF16, tag=nm, name=nm)
             for nm in ("wp1", "wp2")}
    # fp32 SBUF accumulators for the two output projections
    acc = {br: big.tile([128, MC, PROJ], F32, tag=f"acc{br}", name=f"acc{br}")
           for br in range(2)}

    # ---- input DMA: chunked, split across the two HWDGE queues ----
    # Critical path to the first score slot: q1 pair-0 (all of xT) and
    # k2 pair-0 (all of yT).  xT/yT are chunked so the pair-0 qk matmuls
    # can run chunk-outer as the data lands; wv1 (whole) arrives in time
    # for the v lt0/lt1 groups that the first pv slots need.
    nc.scalar.dma_start(out=w_bf["wq1"][:, 0], in_=ins["wq1"][:, 0])
    nc.scalar.dma_start(out=w_bf["wk2"][:, 0], in_=ins["wk2"][:, 0])
    for c in range(KC):
        nc.sync.dma_start(out=xT[:, c, :], in_=ins["xT"][:, c, :])
    nc.scalar.dma_start(out=w_bf["wv1"], in_=ins["wv1"])
    for c in range(KC):
        nc.sync.dma_start(out=yT[:, c, :], in_=ins["yT"][:, c, :])
    nc.sync.dma_start(out=w_bf["wq1"][:, 1:PAIRS], in_=ins["wq1"][:, 1:PAIRS])
    nc.sync.dma_start(out=w_bf["wk2"][:, 1:PAIRS], in_=ins["wk2"][:, 1:PAIRS])
    nc.scalar.dma_start(out=wp_bf["wp1"], in_=ins["wp1"])
    nc.sync.dma_start(out=w_bf["wq2"], in_=ins["wq2"])
    nc.sync.dma_start(out=w_bf["wk1"], in_=ins["wk1"])
    nc.scalar.dma_start(out=w_bf["wv2"], in_=ins["wv2"])
    nc.scalar.dma_start(out=wp_bf["wp2"], in_=ins["wp2"])

    qk = {}     # (name, pair) -> [128, L] bf16 (chan-major q^T / k^T)
    vaug = {}   # (pair, branch) -> [128, MC, 130] bf16, ones at cols 64/129
    onorm = {}  # (pair, branch) -> [128, L] bf16 normalized O^T

    for br in range(2):
        for p in range(PAIRS):
            va = big.tile([128, MC, 130], BF16, tag=f"va_{p}_{br}",
                          name=f"va_{p}_{br}")
            nc.vector.memset(va[:, :, 64:65], 1.0)
            nc.vector.memset(va[:, :, 129:130], 1.0)
            vaug[(p, br)] = va
            onorm[(p, br)] = big.tile([128, L], BF16, tag=f"on_{p}_{br}",
                                      name=f"on_{p}_{br}")
    for nm in ("q1", "k2", "q2", "k1"):
        for p in range(PAIRS):
            qk[(nm, p)] = big.tile([128, L], BF16, tag=f"{nm}_{p}",
                                   name=f"{nm}_{p}")

    def copy_v_out(br, lt, mm):
        for p in range(PAIRS):
            va = vaug[(p, br)]
            nc.vector.tensor_copy(out=va[:, lt, 0:64],
                                  in_=mm[:, p * 128:p * 128 + 64])
            nc.vector.tensor_copy(out=va[:, lt, 65:129],
                                  in_=mm[:, p * 128 + 64:(p + 1) * 128])
    def emit_qk_half(nm, p, lw):
        # one 512-query half of a q/k tensor-pair: 8 accumulating matmuls
        dst = qk[(nm, p)]
        wt = w_bf["w" + nm]
        src = xT if nm in ("q1", "k1") else yT
        lsl = slice(lw * LWIN, (lw + 1) * LWIN)
        mm = fl_ps.tile([128, 512], F32, tag="fl", name="fl")
        for c in range(KC):
            nc.tensor.matmul(mm, wt[:, p, c, :], src[:, c, lsl],
                             start=(c == 0), stop=(c == KC - 1))
        nc.vector.tensor_copy(out=dst[:, lsl], in_=mm)

    def emit_v_group(br, lt):
        wt = w_bf["wv1" if br == 0 else "wv2"]
        src = xT if br == 0 else yT
        mm = fl_ps.tile([128, 512], F32, tag="fl", name="fl")
        for c in range(KC):
            nc.tensor.matmul(mm, src[:, c, lt * 128:(lt + 1) * 128],
                             wt[:, c, :], start=(c == 0), stop=(c == KC - 1))
        copy_v_out(br, lt, mm)

    # ---- start phase ----
    # q1 pair-0 accumulates chunk-outer as xT chunks land (this is the
    # HAM-cold window: 2 matmuls per chunk matches the DMA rate), then
    # k2 pair-0 chunk-outer over yT with the v lt0/lt1 groups interleaved
    # to cover yT's DMA pacing.
    st_q = sc_ps.tile([128, 1024], F32, tag="st", name="st_q")
    for c in range(KC):
        for lw in range(NLW):
            lsl = slice(lw * LWIN, (lw + 1) * LWIN)
            nc.tensor.matmul(st_q[:, lsl], w_bf["wq1"][:, 0, c, :],
                             xT[:, c, lsl], start=(c == 0), stop=(c == KC - 1))
    nc.vector.tensor_copy(out=qk[("q1", 0)][:, 0:512], in_=st_q[:, 0:512])
    nc.vector.tensor_copy(out=qk[("q1", 0)][:, 512:1024], in_=st_q[:, 512:1024])

    st_k = sc_ps.tile([128, 1024], F32, tag="st", name="st_k")
    for c in range(KC):
        for lw in range(NLW):
            lsl = slice(lw * LWIN, (lw + 1) * LWIN)
            nc.tensor.matmul(st_k[:, lsl], w_bf["wk2"][:, 0, c, :],
                             yT[:, c, lsl], start=(c == 0), stop=(c == KC - 1))
        if c == 3:
            emit_v_group(0, 0)
        if c == 6:
            emit_v_group(0, 1)
    nc.vector.tensor_copy(out=qk[("k2", 0)][:, 0:512], in_=st_k[:, 0:512])
    nc.vector.tensor_copy(out=qk[("k2", 0)][:, 512:1024], in_=st_k[:, 512:1024])

    def emit_proj_pair(br, pp, lt):
        # one 128-row chunk of the projection for head-pair pp of branch br,
        # accumulated into the SBUF accumulator; DMA out after the last pair.
        wt = wp_bf[("wp1", "wp2")[br]]
        tsl = slice(lt * 128, (lt + 1) * 128)
        mm = fl_ps.tile([128, 512], F32, tag="fl", name="fl")
        nc.tensor.matmul(mm[:, 0:PROJ], onorm[(pp, br)][:, tsl],
                         wt[:, pp, :], start=True, stop=True)
        a = acc[br]
        if pp == 0:
            nc.vector.tensor_copy(out=a[:, lt, :], in_=mm[:, 0:PROJ])
        else:
            nc.vector.tensor_add(out=a[:, lt, :], in0=a[:, lt, :],
                                 in1=mm[:, 0:PROJ])
        if pp == PAIRS - 1:
            out_r = outs[("p1", "p2")[br]].rearrange("(i p) n -> p i n", p=128)
            nc.sync.dma_start(out=out_r[:, lt, :], in_=a[:, lt, :])

    # ---- attention pipeline pieces ----
    def emit_score(br, p, lw, mc):
        qT = qk[("q1", p)] if br == 0 else qk[("q2", p)]
        kT = qk[("k2", p)] if br == 0 else qk[("k1", p)]
        msl = slice(mc * 128, (mc + 1) * 128)
        lsl = slice(lw * LWIN, (lw + 1) * LWIN)
        st = sc_ps.tile([128, 1024], F32, tag="st", name="st")
        nc.tensor.matmul(st[:, 0:512], kT[0:64, msl], qT[0:64, lsl],
                         start=True, stop=True)
        nc.tensor.matmul(st[:, 512:1024], kT[64:128, msl], qT[64:128, lsl],
                         start=True, stop=True)
        e = ep.tile([128, 1024], BF16, tag="E", name="E")
        nc.scalar.activation(out=e, in_=st, func=EXP, scale=-SCALE)
        return e

    def normalize(p, br, lw, head, pv):
        # pv: [65, 512] psum slice; row 64 is the softmax denominator.
        on = onorm[(p, br)]
        lsl = slice(lw * LWIN, (lw + 1) * LWIN)
        ssum = nrm.tile([1, 512], F32, tag="ssum", name="ssum")
        nc.vector.tensor_copy(out=ssum, in_=pv[64:65, :])
        rrow = nrm.tile([1, 512], F32, tag="rrow", name="rrow")
        nc.vector.reciprocal_approx_fast(out=rrow, in_=ssum)
        pvo = nrm.tile([64, 512], F32, tag="pvo", name="pvo")
        nc.vector.tensor_copy(out=pvo, in_=pv[0:64, :])
        rb = nrm.tile([64, 512], F32, tag="rb", name="rb")
        nc.gpsimd.partition_broadcast(rb, rrow)
        nc.vector.tensor_mul(out=on[head * 64:(head + 1) * 64, lsl],
                             in0=pvo, in1=rb)

    def emit_pv(br, p, lw, mc, e, pvt):
        va = vaug[(p, br)]
        pvA = pvt[:, 0:512]
        pvB = pvt[:, 512:1024]
        nc.tensor.matmul(pvA, va[:, mc, 0:65], e[:, 0:512],
                         start=(mc == 0), stop=(mc == MC - 1))
        nc.tensor.matmul(pvB, va[:, mc, 65:130], e[:, 512:1024],
                         start=(mc == 0), stop=(mc == MC - 1))
        if mc == MC - 1:
            normalize(p, br, lw, 0, pvA)
            normalize(p, br, lw, 1, pvB)

    # ---- filler schedule: (due_slot, group_fn) ----
    # v chunks for br0 (lt 2..7) are needed at slots mc+LAG; pair p's q/k
    # before slot 16p; branch-1 prerequisites before slot 64.  Projections
    # for pair pp of branch br become ready after slot 16*(4*br+pp)+16 plus
    # normalize latency; due ~4 slots later.
    fillers = []
    for lt in range(2, MC):
        fillers.append((lt - 2, lambda lt=lt: emit_v_group(0, lt)))
    for p in (1, 2, 3):
        for lw in range(NLW):
            fillers.append((16 * p - 12 + 2 * lw,
                            lambda p=p, lw=lw: emit_qk_half("q1", p, lw)))
            fillers.append((16 * p - 8 + 2 * lw,
                            lambda p=p, lw=lw: emit_qk_half("k2", p, lw)))
    for lt in range(MC):
        fillers.append((30 + 2 * lt, lambda lt=lt: emit_v_group(1, lt)))
    for lw in range(NLW):
        fillers.append((48 + 2 * lw,
                        lambda lw=lw: emit_qk_half("q2", 0, lw)))
        fillers.append((54 + 2 * lw,
                        lambda lw=lw: emit_qk_half("k1", 0, lw)))
    for p in (1, 2, 3):
        for lw in range(NLW):
            fillers.append((64 + 16 * p - 12 + 2 * lw,
                            lambda p=p, lw=lw: emit_qk_half("q2", p, lw)))
            fillers.append((64 + 16 * p - 8 + 2 * lw,
                            lambda p=p, lw=lw: emit_qk_half("k1", p, lw)))
    # projections: pair pp of branch br completes at slot 16*(4*br+pp)+16
    for br in range(2):
        for pp in range(PAIRS):
            base = 16 * (4 * br + pp) + 21
            for lt in range(MC):
                fillers.append((base + lt,
                                lambda br=br, pp=pp, lt=lt:
                                emit_proj_pair(br, pp, lt)))
    fillers.sort(key=lambda t: t[0])
    fidx = [0]

    def pump(slot):
        while fidx[0] < len(fillers) and fillers[fidx[0]][0] <= slot:
            fillers[fidx[0]][1]()
            fidx[0] += 1

    # ---- main pipeline over attention slots (2-slot batches) ----
    slots = [(br, p, lw, mc)
             for br in range(2) for p in range(PAIRS)
             for lw in range(NLW) for mc in range(MC)]
    e_store = {}
    pv_tiles = {}

    def do_pv(j):
        brj, pj, lwj, mcj = slots[j]
        if mcj == 0:
            pv_tiles[(brj, pj, lwj)] = pv_ps.tile([65, 1024], F32, tag="pv",
                                                  name="pv")
        emit_pv(brj, pj, lwj, mcj, e_store.pop(j), pv_tiles[(brj, pj, lwj)])

    for i0 in range(0, len(slots), 2):
        for i in (i0, i0 + 1):
            br, p, lw, mc = slots[i]
            e_store[i] = emit_score(br, p, lw, mc)
        pump(i0 // 1)
        for i in (i0, i0 + 1):
            j = i - LAG
            if j >= 0:
                do_pv(j)
    for j in (len(slots) - LAG, len(slots) - 1):
        do_pv(j)
    pump(10 ** 9)


def build():
    nc = bacc.Bacc("TRN2", target_bir_lowering=False, debug=False,
                   num_devices=NCORES)
    ins = {}
    for nm in ("xT", "yT"):
        ins[nm] = nc.dram_tensor(nm, [128, KC, L], BF16,
                                 kind="ExternalInput").ap()
    for nm in ("wq1", "wk1", "wq2", "wk2"):
        ins[nm] = nc.dram_tensor(nm, [128, PAIRS, KC, 128], BF16,
                                 kind="ExternalInput").ap()
    for nm in ("wv1", "wv2"):
        ins[nm] = nc.dram_tensor(nm, [128, KC, 512], BF16,
                                 kind="ExternalInput").ap()
    for nm in ("wp1", "wp2"):
        ins[nm] = nc.dram_tensor(nm, [128, PAIRS, PROJ], BF16,
                                 kind="ExternalInput").ap()
    outs = {}
    for nm in ("p1", "p2"):
        outs[nm] = nc.dram_tensor(nm, [L, PROJ], F32, kind="ExternalOutput").ap()
    with tile.TileContext(nc) as tc:
        with ExitStack() as ctx:
            _build_body(nc, tc, ins, outs, ctx)
    nc.compile()
    return nc


_NC_CACHE = None


def _get_nc():
    global _NC_CACHE
    if _NC_CACHE is None:
        _NC_CACHE = build()
    return _NC_CACHE


def _to_cmaj(a):
    """[DIM, N] fp32 -> [128, DIM//128, N] bf16 (contraction chan-major)."""
    n = a.shape[1]
    return np.ascontiguousarray(
        a.reshape(KC, 128, n).transpose(1, 0, 2)).astype(ml_dtypes.bfloat16)


def _to_cmaj_pairs(a):
    """[DIM, 512] fp32 -> [128, PAIRS, KC, 128] bf16 (pair-major)."""
    # a[dim_in, 4*128 out chans]; chan-major over dim_in then split pairs
    t = a.reshape(KC, 128, PAIRS, 128).transpose(1, 2, 0, 3)
    return np.ascontiguousarray(t).astype(ml_dtypes.bfloat16)


def make_in_maps(x, y, w_qkv1, w_qkv2, w_p1, w_p2):
    """Shard the full inputs: core c -> batch c//2, head-slice (c%2)*8."""
    in_maps = []
    for c in range(NCORES):
        b, half = divmod(c, 2)
        c0 = half * 512  # channel offset of this core's 8 heads
        m = {
            "xT": _to_cmaj(np.asarray(x[b]).T),
            "yT": _to_cmaj(np.asarray(y[b]).T),
            "wp1": np.ascontiguousarray(
                np.ascontiguousarray(w_p1[c0:c0 + 512, :])
                .reshape(PAIRS, 128, PROJ).transpose(1, 0, 2))
                .astype(ml_dtypes.bfloat16),
            "wp2": np.ascontiguousarray(
                np.ascontiguousarray(w_p2[c0:c0 + 512, :])
                .reshape(PAIRS, 128, PROJ).transpose(1, 0, 2))
                .astype(ml_dtypes.bfloat16),
        }
        for wsrc, names in ((w_qkv1, ("wq1", "wk1", "wv1")),
                            (w_qkv2, ("wq2", "wk2", "wv2"))):
            for j, nm in enumerate(names):
                base = j * DIM + c0
                wslice = np.ascontiguousarray(wsrc[:, base:base + 512])
                if nm.startswith("wv"):
                    m[nm] = _to_cmaj(wslice)
                else:
                    m[nm] = _to_cmaj_pairs(wslice)
        in_maps.append(m)
    return in_maps


def run_cores(in_maps, trace=False, trace_cores=None):
    nc = _get_nc()
    return run_bass_kernel_spmd(nc, in_maps, list(range(NCORES)),
                                trace=trace, trace_cores=trace_cores)


def kernel(x, y, w_qkv1, w_qkv2, w_p1, b_p1, w_p2, b_p2):
    x = np.asarray(x, dtype=np.float32)
    y = np.asarray(y, dtype=np.float32)
    in_maps = make_in_maps(x, y, np.asarray(w_qkv1), np.asarray(w_qkv2),
                           np.asarray(w_p1), np.asarray(w_p2))
    res = run_cores(in_maps).results
    out1 = np.stack([res[2 * b]["p1"] + res[2 * b + 1]["p1"] for b in range(4)])
    out2 = np.stack([res[2 * b]["p2"] + res[2 * b + 1]["p2"] for b in range(4)])
    out1 += np.asarray(b_p1, dtype=np.float32)
    out2 += np.asarray(b_p2, dtype=np.float32)
    return out1, out2
